# revision 1
# baseline (speedup 1.0000x reference)
"""Causal self-attention (B=2, T=2048, C=1024, H=16, D=64) on 8 trn2 NeuronCores.

Sharding: data-parallel over batch (2) x tensor-parallel over heads (4 groups
of 4 heads). Each core computes qkv projection for its 4 heads, causal
attention, and a partial output projection; the host sums the 4 TP partials
per batch (the tensor-parallel all-reduce) and stacks the batches.

Device kernel layout notes:
  - all matmuls bf16 (inputs cast on host), fp32 PSUM accumulation
  - qT/kT stored [d, T] pairs (2 heads stacked on partitions) so the
    S^T matmul (contraction over d=64) row-packs two heads on the PE array
  - S^T tiles [tk=128, tq<=2048] -> exp on ScalarE (scale=1/8 folded in,
    no max subtraction: |S*scale| <= ~8 so exp is safe in fp32/bf16)
  - causal mask: only lower-triangular tk-tiles are computed; the diagonal
    128x128 block is masked multiplicatively after exp
  - A^T = [v|1]^T @ P^T accumulated over tk tiles; row 64 is the softmax
    denominator (ones column trick); normalization by 1/rowsum is applied
    with a fast-reciprocal + partition-broadcast DMA
  - output projection uses yT tiles stationary, w_proj moving; result DMAd
    straight from an SBUF staging tile to HBM in fp32
"""

import numpy as np
import ml_dtypes

B, T, C = 2, 2048, 1024
N_HEAD, D = 16, 64
N_CORES = 8
TPG = 4  # tensor-parallel groups (head groups)
HL = 4  # heads per core
CT = C // 128  # 8 contraction tiles for the projections
TTN = T // 128  # 16 token tiles

_BF16 = ml_dtypes.bfloat16

_CACHE = {}


def _split_sync_waits(nc):
    """walrus in this container rejects >1 semaphore wait per instruction
    ("Too many sync wait commands" in setupSyncWait). Hoist extra waits onto
    same-engine NOPs inserted immediately before the instruction — engines
    execute their stream in order, so semantics are preserved."""
    import concourse.mybir as mybir

    k = 0
    for f in nc.m.functions:
        for bb in f.blocks:
            out = []
            for inst in bb.instructions:
                si = inst.sync_info
                if si is not None and len(si.on_wait) > 1:
                    waits = list(si.on_wait)
                    assert inst.engine != mybir.EngineType.Unassigned, inst
                    for w in waits[:-1]:
                        nop = mybir.InstNoOp(name=f"WSPLIT-{k}")
                        k += 1
                        nop.engine = inst.engine
                        nop.sync_info = mybir.SyncInfo(on_wait=[w], on_update=[])
                        out.append(nop)
                    inst.sync_info = mybir.SyncInfo(
                        on_wait=[waits[-1]], on_update=list(si.on_update)
                    )
                out.append(inst)
            bb.instructions = out


def _build_nc(reps=1):
    import concourse.bass as bass
    import concourse.mybir as mybir
    import concourse.tile as tile
    from concourse.masks import make_upper_triangular
    from contextlib import ExitStack

    bf16 = mybir.dt.bfloat16
    f32 = mybir.dt.float32
    f32r = mybir.dt.float32r
    Exp = mybir.ActivationFunctionType.Exp

    nc = bass.Bass("TRN2", target_bir_lowering=False, debug=False, num_devices=N_CORES)

    xT_d = nc.declare_dram_parameter("xT", [C, T], bf16, isOutput=False)
    wqk_d = nc.declare_dram_parameter("wqk", [C, 4 * 128], bf16, isOutput=False)
    wv_d = nc.declare_dram_parameter("wv", [C, HL * D], bf16, isOutput=False)
    wpr_d = nc.declare_dram_parameter("wpr", [HL * D, C], bf16, isOutput=False)
    out_d = nc.declare_dram_parameter("out", [T, C], f32, isOutput=True)

    # tq extent / flat offset for each tk tile (only tq >= tk tile start kept)
    ext = [T - 128 * i for i in range(TTN)]

    with ExitStack() as ctx:
        tc = ctx.enter_context(tile.TileContext(nc))
        pool_w = ctx.enter_context(tc.tile_pool(name="w", bufs=1))
        pool_qkvo = ctx.enter_context(tc.tile_pool(name="qkvo", bufs=1))
        pool_pt = ctx.enter_context(tc.tile_pool(name="pt", bufs=23))
        pool_r = ctx.enter_context(tc.tile_pool(name="r", bufs=3))
        pool_ost = ctx.enter_context(tc.tile_pool(name="ost", bufs=3))
        ps_mm = ctx.enter_context(tc.tile_pool(name="psmm", bufs=2, space="PSUM"))
        ps_st = ctx.enter_context(tc.tile_pool(name="psst", bufs=2, space="PSUM"))
        ps_at = ctx.enter_context(tc.tile_pool(name="psat", bufs=2, space="PSUM"))

        for rep in range(reps):
            # ---- load weights + constants ----
            wqk_sb = pool_w.tile([128, CT, 4 * 128], bf16)
            nc.sync.dma_start(
                out=wqk_sb[:],
                in_=wqk_d[:, :].rearrange("(ct p) n -> p ct n", p=128),
            )
            wv_sb = pool_w.tile([128, CT, HL * D], bf16)
            nc.sync.dma_start(
                out=wv_sb[:], in_=wv_d[:, :].rearrange("(ct p) n -> p ct n", p=128)
            )
            wpr_sb = pool_w.tile([128, 2, C], bf16)
            nc.sync.dma_start(
                out=wpr_sb[:], in_=wpr_d[:, :].rearrange("(ci p) n -> p ci n", p=128)
            )
            mask_sb = pool_w.tile([128, 128], bf16)
            make_upper_triangular(nc, mask_sb[:], val=1.0, diag=True)
            ones_col = pool_w.tile([1, 64], f32)
            nc.vector.memset(ones_col[:], 1.0)
            ones_col_r = pool_w.tile([1, 64], f32r)
            with nc.allow_low_precision(reason="f32r constant"):
                nc.vector.tensor_copy(ones_col_r[:], ones_col[:])

            xT_pool_ctx = ExitStack()
            pool_x = xT_pool_ctx.enter_context(tc.tile_pool(name=f"x{rep}", bufs=1))
            xT_sb = pool_x.tile([128, CT, T], bf16)
            for ct in range(CT):
                nc.sync.dma_start(
                    out=xT_sb[:, ct, :], in_=xT_d[128 * ct : 128 * (ct + 1), :]
                )

            # ---- phase 1: q/k projections (weights stationary) ----
            # group g = 2*pair + (0:q, 1:k); output rows 0-63 = head 2*pair,
            # rows 64-127 = head 2*pair+1
            qT = pool_qkvo.tile([128, 2, T], bf16)
            kT = pool_qkvo.tile([128, 2, T], bf16)
            for pair in range(2):
                for qk in range(2):
                    g = 2 * pair + qk
                    dst = qT if qk == 0 else kT
                    for j in range(4):
                        ps = ps_mm.tile([128, 512], f32, tag="mm")
                        for ct in range(CT):
                            nc.tensor.matmul(
                                ps[:],
                                wqk_sb[:, ct, 128 * g : 128 * (g + 1)],
                                xT_sb[:, ct, 512 * j : 512 * (j + 1)],
                                start=(ct == 0),
                                stop=(ct == CT - 1),
                            )
                        nc.vector.tensor_copy(dst[:, pair, 512 * j : 512 * (j + 1)], ps[:])

            # ---- phase 2: v projection (xT tiles stationary) + ones column ----
            v_sb = pool_qkvo.tile([128, TTN, HL, D + 1], bf16)
            nc.vector.memset(v_sb[:, :, :, D : D + 1], 1.0)
            for tt in range(TTN):
                ps = ps_mm.tile([128, 512], f32, tag="mm")
                for ct in range(CT):
                    nc.tensor.matmul(
                        ps[:, 0 : HL * D],
                        xT_sb[:, ct, 128 * tt : 128 * (tt + 1)],
                        wv_sb[:, ct, :],
                        start=(ct == 0),
                        stop=(ct == CT - 1),
                    )
                nc.vector.tensor_copy(
                    v_sb[:, tt, :, 0:D],
                    ps[:, 0 : HL * D].rearrange("p (h d) -> p h d", d=D),
                )

            # ---- phase 3: attention, head pairs interleaved so the two
            # halves' K=64 S^T matmuls (row groups 0-63 / 64-127) run
            # concurrently on the PE array ----
            yT = pool_qkvo.tile([128, 2, T], bf16)
            for pair in range(2):
                pt2 = [[], []]
                for i in range(TTN):
                    E = ext[i]
                    pts = [pool_pt.tile([128, E], bf16, tag="pt", name=f"pt{_h}") for _h in range(2)]
                    pt2[0].append(pts[0])
                    pt2[1].append(pts[1])
                    pos = 0
                    while pos < E:
                        fd = min(1024, E - pos)
                        pss = [ps_st.tile([128, 1024], f32, tag="st", name=f"st{_h}") for _h in range(2)]
                        for n0 in range(0, fd, 512):
                            w_ = min(512, fd - n0)
                            for half in range(2):
                                pb = 64 * half
                                nc.tensor.matmul(
                                    pss[half][:, n0 : n0 + w_],
                                    kT[pb : pb + 64, pair, 128 * i : 128 * (i + 1)],
                                    qT[pb : pb + 64, pair, 128 * i + pos + n0 : 128 * i + pos + n0 + w_],
                                    start=True,
                                    stop=True,
                                )
                        for half in range(2):
                            nc.scalar.activation(
                                pts[half][:, pos : pos + fd],
                                pss[half][:, 0:fd],
                                Exp,
                                scale=0.125,
                            )
                        pos += fd
                    # causal mask on the diagonal 128x128 block
                    for half in range(2):
                        nc.vector.tensor_mul(
                            pts[half][:, 0:128], pts[half][:, 0:128], mask_sb[:]
                        )

                for half, j in [(hf, jj) for hf in range(2) for jj in range(4)]:
                    h = 2 * pair + half
                    pb = 64 * half
                    pt_tiles = pt2[half]
                    ps_a = ps_at.tile([128, 512], f32, tag="at")
                    ntiles = 4 * j + 4
                    for i in range(ntiles):
                        lo = max(512 * j, 128 * i)  # first tq col this tile covers
                        nc.tensor.matmul(
                            ps_a[0 : D + 1, lo - 512 * j : 512],
                            v_sb[:, i, h, :],
                            pt_tiles[i][:, lo - 128 * i : lo - 128 * i + (512 * (j + 1) - lo)],
                            start=(i == 0),
                            stop=(i == ntiles - 1),
                        )
                    # softmax denominator -> reciprocal -> broadcast -> normalize.
                    # reciprocal is 8 cyc/elem on one lane, so reshape the row to
                    # a [128, 4] column block via DMA first (and back after).
                    r_row = pool_r.tile([1, 512], f32, tag="rrow")
                    nc.vector.tensor_copy(r_row[:], ps_a[D : D + 1, :])
                    r_col = pool_r.tile([128, 4], f32, tag="rcol")
                    nc.sync.dma_start(out=r_col[:, :], in_=r_row[0:1, :])
                    r_colr = pool_r.tile([128, 4], f32r, tag="rcolr")
                    with nc.allow_low_precision(reason="f32r reciprocal, not an accumulation"):
                        nc.vector.reciprocal(r_colr[:], r_col[:])
                    r_rec = pool_r.tile([1, 512], f32r, tag="rrec")
                    nc.sync.dma_start(out=r_rec[0:1, :], in_=r_colr[:, :])
                    # replicate 1/rowsum across 64 partitions: ones[1,64].T @ r
                    # (float32r runs the PE at 1 cyc/row vs fp32's 4)
                    r_ps = ps_mm.tile([64, 512], f32, tag="mm")
                    nc.tensor.matmul(
                        r_ps[:], ones_col_r[0:1, :], r_rec[0:1, :], start=True, stop=True
                    )
                    r_bc = pool_r.tile([64, 512], f32, tag="rbc")
                    nc.vector.tensor_copy(r_bc[:], r_ps[:])
                    nc.vector.tensor_mul(
                        yT[pb : pb + 64, pair, 512 * j : 512 * (j + 1)],
                        ps_a[0:D, :],
                        r_bc[:],
                    )

            xT_pool_ctx.close()

            # ---- phase 4: output projection (yT stationary, w_proj moving) ----
            for tt in range(TTN):
                for co in range(2):
                    ps = ps_mm.tile([128, 512], f32, tag="mm")
                    for ci in range(2):
                        nc.tensor.matmul(
                            ps[:],
                            yT[:, ci, 128 * tt : 128 * (tt + 1)],
                            wpr_sb[:, ci, 512 * co : 512 * (co + 1)],
                            start=(ci == 0),
                            stop=(ci == 1),
                        )
                    so = pool_ost.tile([128, 512], f32, tag="ostage")
                    nc.vector.tensor_copy(so[:], ps[:])
                    nc.sync.dma_start(
                        out=out_d[128 * tt : 128 * (tt + 1), 512 * co : 512 * (co + 1)],
                        in_=so[:],
                    )

    _split_sync_waits(nc)
    return nc


def _get_nc():
    if "nc" not in _CACHE:
        _CACHE["nc"] = _build_nc()
    return _CACHE["nc"]


def _shard_inputs(x, w_qkv, w_proj):
    """Host-side shard prep. Returns in_maps for cores 0..7; core = b*4 + hg."""
    xT = [np.ascontiguousarray(x[b].T).astype(_BF16) for b in range(B)]
    in_maps = []
    wq = w_qkv[:, 0:C]
    wk = w_qkv[:, C : 2 * C]
    wv = w_qkv[:, 2 * C : 3 * C]
    per_group = []
    for hg in range(TPG):
        heads = [hg * HL + i for i in range(HL)]
        qcols = [wq[:, h * D : (h + 1) * D] for h in heads]
        kcols = [wk[:, h * D : (h + 1) * D] for h in heads]
        vcols = [wv[:, h * D : (h + 1) * D] for h in heads]
        wqk_hg = np.concatenate(
            [qcols[0], qcols[1], kcols[0], kcols[1], qcols[2], qcols[3], kcols[2], kcols[3]],
            axis=1,
        ).astype(_BF16)
        wv_hg = np.concatenate(vcols, axis=1).astype(_BF16)
        wpr_hg = np.ascontiguousarray(
            w_proj[hg * HL * D : (hg + 1) * HL * D, :]
        ).astype(_BF16)
        per_group.append((wqk_hg, wv_hg, wpr_hg))
    for b in range(B):
        for hg in range(TPG):
            wqk_hg, wv_hg, wpr_hg = per_group[hg]
            in_maps.append({"xT": xT[b], "wqk": wqk_hg, "wv": wv_hg, "wpr": wpr_hg})
    return in_maps


def kernel(x, w_qkv, w_proj):
    from concourse.bass_utils import run_bass_kernel_spmd

    x = np.asarray(x, dtype=np.float32)
    w_qkv = np.asarray(w_qkv, dtype=np.float32)
    w_proj = np.asarray(w_proj, dtype=np.float32)

    nc = _get_nc()
    in_maps = _shard_inputs(x, w_qkv, w_proj)
    res = run_bass_kernel_spmd(nc, in_maps, list(range(N_CORES)))

    out = np.zeros((B, T, C), dtype=np.float32)
    for b in range(B):
        acc = np.zeros((T, C), dtype=np.float32)
        for hg in range(TPG):
            acc += res.results[b * TPG + hg]["out"]
        out[b] = acc
    return out

